# revision 4
# baseline (speedup 1.0000x reference)
"""MoE (top-4 of 16 experts, SwiGLU FFN) on 8 Trainium2 NeuronCores.

Strategy: expert parallelism. The router (x @ Wr, softmax, top-4) is 0.26% of
the FLOPs and runs on host; tokens are gathered per expert on host (the
"all-to-all dispatch"), each core runs the dense SwiGLU FFN for its 2 experts
on its gathered tokens in bf16 (fp32 PSUM accumulation), and the host
scatter-adds the weighted expert outputs back ("combine").

v2 layout: stage B is computed transposed (y^T = wd^T @ h, tokens as the
moving free dim), so no token count needs 128-alignment anywhere and the
combine weight is applied by the vector engine against a host-prepared
broadcast tile (the scalar engine only runs the silus). Startup DMAs are
ordered critical-slice-first (wg f-quarter 0 + a 128-token lead chunk), the
PE is pre-warmed with dummy matmuls during the initial DMA wait (HAM clock
gate), stage B of the first two chunks is deferred one chunk so the PE never
waits on the down-proj weights, and y writeback alternates two DMA queues.

Shapes (hardcoded): B=4, S=1024, D=1024, E=16, F=512, TOPK=4. N = B*S = 4096.
All DRAM arrays are pre-tiled on host so every DMA is partition-contiguous
(128/256 descriptors of >=1 KiB).
"""

import numpy as np
import ml_dtypes

import concourse.bass as bass
import concourse.bacc as bacc
import concourse.tile as tile
from concourse import bass_utils, mybir

B, S, D = 4, 1024, 1024
E, F, TOPK = 16, 512, 4
N = B * S
NCORES = 8
EPC = E // NCORES  # experts per core
P = 128
DT = D // P  # 8
FT = F // P  # 4
TCH = 512    # token chunk (matmul moving free dim; one PSUM bank in fp32)
NWARM = 18   # PE pre-warm matmuls (N=128, run during startup DMA wait)

BF16 = ml_dtypes.bfloat16

_program_cache: dict[tuple, object] = {}


# ---------------------------------------------------------------- host router
def _route(xf: np.ndarray, Wr: np.ndarray):
    """Top-4 expert ids + renormalized weights per token.

    Renormalized top-k softmax weights == softmax over just the top-k logits,
    so the full softmax denominator is never needed.
    """
    logits = xf @ Wr  # [N, E] fp32
    idx = np.argpartition(-logits, TOPK - 1, axis=1)[:, :TOPK]  # [N, K]
    lt = np.take_along_axis(logits, idx, axis=1)
    lt = lt - lt.max(axis=1, keepdims=True)
    ex = np.exp(lt)
    w = ex / ex.sum(axis=1, keepdims=True)
    return idx, w.astype(np.float32)


def _chunks_of(C: int, lead: int = 0, small_tail: bool = False):
    """Split C tokens into chunks of <= TCH, as even as possible (no
    alignment requirement). An optional `lead` chunk lets the PE start before
    a full token block has arrived; `small_tail` puts a 128-token chunk last
    so the final drain + writeback is short."""
    sizes = []
    if lead and C >= lead + P:
        sizes.append(lead)
        C -= lead
    tail = 0
    if small_tail and C >= P + P:
        tail = P
        C -= tail
    n = -(-C // TCH)
    per = C // n
    extra = C - per * n
    sizes += [per + 1] * extra + [per] * (n - extra)
    if tail:
        sizes.append(tail)
    out, t0 = [], 0
    for sz in sizes:
        out.append((t0, sz))
        t0 += sz
    return out


def _layout(caps: tuple):
    """Flat chunk descriptors shared by program build and host prep.

    Each entry: (slot, t0, tch, xt_off, gcol, y_off)
      xt_off: element offset of this chunk's [P, DT, tch] block in flat xt
      gcol:   global gathered-token column (cw / slot offset), exact
      y_off:  element offset of this chunk's 4 [P, 2*tch] pair blocks in y
    """
    slot_chunks = [
        _chunks_of(C, lead=P if s == 0 else 0, small_tail=(s == EPC - 1))
        for s, C in enumerate(caps)
    ]
    flat = []
    xoff = yoff = 0
    gbase = 0
    for s, C in enumerate(caps):
        for (t0, tch) in slot_chunks[s]:
            flat.append((s, t0, tch, xoff, gbase + t0, yoff))
            xoff += P * DT * tch
            yoff += (DT // 2) * P * 2 * tch
        gbase += C
    return flat, xoff, yoff, gbase  # gbase == CTOT


# ---------------------------------------------------------------- device code
def _build_program(caps: tuple):
    """One SPMD program: EPC expert slots with exact capacities caps[s].

    Inputs (per core), all pre-tiled partition-major on host:
      xt  [XTELEMS]             bf16  gathered+transposed tokens, per chunk
                                      [p, d, c] blocks
      wg  [EPC, FT, P, DT, P]   bf16  wg[s, f, p, d, q] = Wg_s[d*128+p, f*128+q]
      wu  [EPC, FT, P, DT, P]   bf16
      wd  [EPC, P, FT, D]       bf16  wd[s, p, f, d] = Wd_s[f*128+p, d]
      cwr [P, CTOT]             f32   combine weight per gathered token,
                                      replicated along partitions
    Output:
      y   [YELEMS]              bf16  per chunk: 4 pair blocks [p, 2, tch]
                                      holding y^T rows (2dp+j)*128+p
    """
    flat, XTELEMS, YELEMS, CTOT = _layout(caps)
    nchunks = len(flat)

    nc = bacc.Bacc("TRN2", target_bir_lowering=False, debug=False)
    bf = mybir.dt.bfloat16
    f32 = mybir.dt.float32

    xt = nc.declare_dram_parameter("xt", [XTELEMS], bf, isOutput=False)
    wg = nc.declare_dram_parameter("wg", [EPC, FT, P, DT, P], bf, isOutput=False)
    wu = nc.declare_dram_parameter("wu", [EPC, FT, P, DT, P], bf, isOutput=False)
    wd = nc.declare_dram_parameter("wd", [EPC, P, FT, D], bf, isOutput=False)
    cwr = nc.declare_dram_parameter("cwr", [P, CTOT], f32, isOutput=False)
    y = nc.declare_dram_parameter("y", [YELEMS], bf, isOutput=True)

    with tile.TileContext(nc) as tc:
        with (
            tc.tile_pool(name="wrm", bufs=1) as wrm,
            tc.tile_pool(name="wpool", bufs=2) as wpool,
            tc.tile_pool(name="cpool", bufs=1) as cpool,
            tc.tile_pool(name="xpool", bufs=3) as xpool,
            tc.tile_pool(name="hpool", bufs=3) as hpool,
            tc.tile_pool(name="spool", bufs=2) as spool,
            tc.tile_pool(name="ypool", bufs=4) as ypool,
            tc.tile_pool(name="psA", bufs=2, space="PSUM") as psA,
            tc.tile_pool(name="psB", bufs=3, space="PSUM") as psB,
        ):
            # --- PE pre-warm: HAM releases the clock gate after ~3.4us of
            # activity; start that clock during the startup DMA wait.
            wt = wrm.tile([P, P], bf, tag="wt")
            nc.gpsimd.memset(wt[:], 0.0)
            for _ in range(NWARM):
                pw = psA.tile([P, TCH], f32, tag="pg")
                nc.tensor.matmul(
                    pw[:, :P], lhsT=wt[:], rhs=wt[:], start=True, stop=True
                )

            wg_sb = [
                wpool.tile([P, FT, DT, P], bf, tag="wg", name=f"wg_sb{s}")
                for s in range(EPC)
            ]
            wu_sb = [
                wpool.tile([P, FT, DT, P], bf, tag="wu", name=f"wu_sb{s}")
                for s in range(EPC)
            ]
            wd_sb = [
                wpool.tile([P, FT, D], bf, tag="wd", name=f"wd_sb{s}")
                for s in range(EPC)
            ]
            cw_sb = cpool.tile([P, CTOT], f32, tag="cw")

            xt_tiles = {}

            def xt_load(k, eng):
                # contiguous SBUF destination (any partial-width dst slice
                # explodes into 1024 descriptors); matmuls use a strided view
                _, _, tch, xo, _, _ = flat[k]
                t = xpool.tile([P, DT * tch], bf, tag="xt")
                src = xt[xo : xo + P * DT * tch].rearrange("(p x) -> p x", p=P)
                eng.dma_start(t[:], src)
                xt_tiles[k] = t.rearrange("p (dt c) -> p dt c", dt=DT)

            # --- startup DMAs, critical-path first. DMA instruction issue
            # costs ~660ns of sequencer time each; the three DMA-capable
            # sequencers (gpsimd / sync / scalar) are used in parallel.
            nc.gpsimd.dma_start(wg_sb[0][:, 0], wg[0, 0])  # first matmul gate
            xt_load(0, nc.sync)                            # 128-token lead
            nc.gpsimd.dma_start(wg_sb[0][:, 1], wg[0, 1])
            nc.sync.dma_start(wg_sb[0][:, 2], wg[0, 2])
            nc.sync.dma_start(wg_sb[0][:, 3], wg[0, 3])
            for f in range(FT):
                nc.scalar.dma_start(wu_sb[0][:, f], wu[0, f])
            xt_load(1, nc.sync)
            if nchunks > 2:
                xt_load(2, nc.sync)
            nc.gpsimd.dma_start(cw_sb[:], cwr[:])          # needed by 1st B
            nc.gpsimd.dma_start(wd_sb[0][:], wd[0])

            # slot-1 weight DMAs are paced into the first two stage-B blocks
            # (below): by then the startup bandwidth hump has drained.
            later_w = [
                (wg_sb[1][:, 0], wg[1, 0]),
                (wg_sb[1][:, 1], wg[1, 1]),
                (wg_sb[1][:, 2], wg[1, 2]),
                (wg_sb[1][:, 3], wg[1, 3]),
                (wu_sb[1][:, 0], wu[1, 0]),
                (wu_sb[1][:, 1], wu[1, 1]),
                (wu_sb[1][:, 2], wu[1, 2]),
                (wu_sb[1][:, 3], wu[1, 3]),
                (wd_sb[1][:], wd[1]),
            ] if EPC > 1 else []

            h_tiles = {}

            def stage_a(k):
                s, t0, tch, _, _, _ = flat[k]
                xt_sb = xt_tiles.pop(k)
                h_sb = hpool.tile([P, FT, TCH], bf, tag="h")
                sgs = []
                for f in range(FT):
                    pg = psA.tile([P, TCH], f32, tag="pg")
                    for d in range(DT):
                        nc.tensor.matmul(
                            pg[:, :tch],
                            lhsT=wg_sb[s][:, f, d, :],
                            rhs=xt_sb[:, d, :tch],
                            start=(d == 0),
                            stop=(d == DT - 1),
                        )
                    sg = spool.tile([P, TCH], f32, tag=f"sg{f}")
                    nc.scalar.activation(
                        sg[:, :tch],
                        pg[:, :tch],
                        mybir.ActivationFunctionType.Silu,
                    )
                    sgs.append(sg)
                for f in range(FT):
                    pu = psA.tile([P, TCH], f32, tag="pu")
                    for d in range(DT):
                        nc.tensor.matmul(
                            pu[:, :tch],
                            lhsT=wu_sb[s][:, f, d, :],
                            rhs=xt_sb[:, d, :tch],
                            start=(d == 0),
                            stop=(d == DT - 1),
                        )
                    nc.vector.tensor_mul(
                        out=h_sb[:, f, :tch],
                        in0=sgs[f][:, :tch],
                        in1=pu[:, :tch],
                    )
                    if f == 1 and k + 3 < nchunks:
                        xt_load(k + 3, nc.sync)
                h_tiles[k] = h_sb

            def stage_b(k, extra_w=()):
                s, t0, tch, _, gcol, yo = flat[k]
                h_sb = h_tiles.pop(k)
                extra_w = list(extra_w)
                for dp in range(DT // 2):
                    y_sb = ypool.tile([P, 2 * TCH], bf, tag="y")
                    for j in range(2):
                        dd = 2 * dp + j
                        pb = psB.tile([P, TCH], f32, tag="pb")
                        for f in range(FT):
                            nc.tensor.matmul(
                                pb[:, :tch],
                                lhsT=wd_sb[s][:, f, dd * P : (dd + 1) * P],
                                rhs=h_sb[:, f, :tch],
                                start=(f == 0),
                                stop=(f == FT - 1),
                            )
                        nc.vector.tensor_mul(
                            out=y_sb[:, j * tch : (j + 1) * tch],
                            in0=pb[:, :tch],
                            in1=cw_sb[:, gcol : gcol + tch],
                        )
                    dst = y[
                        yo + dp * P * 2 * tch : yo + (dp + 1) * P * 2 * tch
                    ].rearrange("(p x) -> p x", p=P)
                    (nc.gpsimd if dp % 2 == 0 else nc.sync).dma_start(
                        dst, y_sb[:, : 2 * tch]
                    )
                    # pace deferred weight loads between y writebacks
                    for _ in range(3):
                        if extra_w:
                            t, srcap = extra_w.pop(0)
                            nc.gpsimd.dma_start(t, srcap)

            # pipeline: defer the first two stage-Bs one chunk so the PE
            # never waits on wd/cwr during the startup bandwidth hump.
            if nchunks >= 2:
                stage_a(0)
                stage_a(1)
                stage_b(0, extra_w=later_w[:5])
                stage_b(1, extra_w=later_w[5:])
                for k in range(2, nchunks):
                    stage_a(k)
                    stage_b(k)
            else:
                stage_a(0)
                stage_b(0, extra_w=later_w)
    nc.compile()
    return nc


def _get_program(caps):
    if caps not in _program_cache:
        _program_cache[caps] = _build_program(caps)
    return _program_cache[caps]


# ------------------------------------------------------------------ profiling
def _ensure_ntff_hook():
    """The container's `antenv` stub lacks `axon_hooks`, so trn_boot's NTFF
    profile hook never gets registered and trace=True degrades to no-op.
    Register the module + ctypes hook at runtime."""
    import sys
    import types

    import antenv

    if "antenv.axon_hooks" not in sys.modules:
        mod = types.ModuleType("antenv.axon_hooks")
        mod._hook = None

        def set_axon_ntff_profile_hook(h):
            mod._hook = h

        def get_axon_ntff_profile_hook():
            return mod._hook

        mod.set_axon_ntff_profile_hook = set_axon_ntff_profile_hook
        mod.get_axon_ntff_profile_hook = get_axon_ntff_profile_hook
        sys.modules["antenv.axon_hooks"] = mod
        antenv.axon_hooks = mod
    mod = sys.modules["antenv.axon_hooks"]
    if mod._hook is None:
        from trn_agent_boot.trn_boot import _ntff_profile_via_ctypes

        mod.set_axon_ntff_profile_hook(
            _ntff_profile_via_ctypes("/opt/axon/libaxon_pjrt.so")
        )


# ---------------------------------------------------------------- entry point
def _run(inputs: dict, trace: bool = False):
    x = np.asarray(inputs["x"], dtype=np.float32)
    Wr = np.asarray(inputs["Wr"], dtype=np.float32)
    Wg = np.asarray(inputs["Wg"], dtype=np.float32)
    Wu = np.asarray(inputs["Wu"], dtype=np.float32)
    Wd = np.asarray(inputs["Wd"], dtype=np.float32)

    xf = x.reshape(N, D)
    idx, w = _route(xf, Wr)

    # group (token, weight) by expert
    flat_e = idx.ravel()
    flat_t = np.repeat(np.arange(N, dtype=np.int64), TOPK)
    flat_w = w.ravel()
    order = np.argsort(flat_e, kind="stable")
    ge, gt, gw = flat_e[order], flat_t[order], flat_w[order]
    counts = np.bincount(ge, minlength=E)
    starts = np.zeros(E + 1, dtype=np.int64)
    np.cumsum(counts, out=starts[1:])

    # global pairing: sort experts by count desc, core c gets ranks (c, 15-c);
    # slot 0 holds the larger one. caps = per-slot max over cores (exact).
    by_size = sorted(range(E), key=lambda e: -counts[e])
    slot_experts = [
        [by_size[c], by_size[E - 1 - c]] for c in range(NCORES)
    ]  # [core][slot] -> expert id
    caps = tuple(
        int(max(counts[slot_experts[c][s]] for c in range(NCORES)))
        for s in range(EPC)
    )
    flat, XTELEMS, YELEMS, CTOT = _layout(caps)
    slot_off = np.cumsum([0] + list(caps))

    xt_all = np.zeros((NCORES, XTELEMS), dtype=BF16)
    cwr_all = np.zeros((NCORES, P, CTOT), dtype=np.float32)
    wg_all = np.zeros((NCORES, EPC, FT, P, DT, P), dtype=BF16)
    wu_all = np.zeros((NCORES, EPC, FT, P, DT, P), dtype=BF16)
    wd_all = np.zeros((NCORES, EPC, P, FT, D), dtype=BF16)

    tok_lists = {}
    for c in range(NCORES):
        cw_row = np.zeros(CTOT, dtype=np.float32)
        for s in range(EPC):
            e = slot_experts[c][s]
            toks = gt[starts[e] : starts[e + 1]]
            tok_lists[(c, s)] = toks
            ne = len(toks)
            cw_row[slot_off[s] : slot_off[s] + ne] = gw[starts[e] : starts[e + 1]]
            # weights, partition-major K tiles
            wg_all[c, s] = (
                Wg[e].astype(BF16).reshape(DT, P, FT, P).transpose(2, 1, 0, 3)
            )
            wu_all[c, s] = (
                Wu[e].astype(BF16).reshape(DT, P, FT, P).transpose(2, 1, 0, 3)
            )
            wd_all[c, s] = Wd[e].astype(BF16).reshape(FT, P, D).transpose(1, 0, 2)
        cwr_all[c] = cw_row[None, :]
        # tokens, transposed + tiled per chunk: [p, d, c] = X[tok, d*128+p]
        for (s, t0, tch, xo, gcol, yo) in flat:
            toks = tok_lists[(c, s)]
            sel = toks[t0 : min(t0 + tch, len(toks))]
            blk = np.zeros((P, DT, tch), dtype=BF16)
            if len(sel):
                blk[:, :, : len(sel)] = (
                    xf[sel]
                    .astype(BF16)
                    .reshape(len(sel), DT, P)
                    .transpose(2, 1, 0)
                )
            xt_all[c, xo : xo + P * DT * tch] = blk.ravel()

    nc = _get_program(caps)
    in_maps = [
        {
            "xt": xt_all[c],
            "wg": wg_all[c],
            "wu": wu_all[c],
            "wd": wd_all[c],
            "cwr": cwr_all[c],
        }
        for c in range(NCORES)
    ]
    kwargs = {}
    if trace:
        _ensure_ntff_hook()
        kwargs = dict(trace=True, trace_cores=list(range(NCORES)))
    res = bass_utils.run_bass_kernel_spmd(
        nc, in_maps, core_ids=list(range(NCORES)), **kwargs
    )

    out = np.zeros((N, D), dtype=np.float32)
    for c in range(NCORES):
        yc = res.results[c]["y"]
        for (s, t0, tch, xo, gcol, yo) in flat:
            toks = tok_lists[(c, s)]
            sel = toks[t0 : min(t0 + tch, len(toks))]
            if not len(sel):
                continue
            blk = yc[yo : yo + (DT // 2) * P * 2 * tch].reshape(DT // 2, P, 2, tch)
            # rows (2*dp + j)*128 + p of y^T  ->  [D, tch]
            yt = blk.transpose(0, 2, 1, 3).reshape(D, tch)
            out[sel] += yt[:, : len(sel)].T.astype(np.float32)
    return out.reshape(B, S, D), res.exec_time_ns


# Pre-register the NTFF hook shim at import: if the grading harness sets
# BASS_TRACE=1, run_bass_kernel_spmd's axon trace path imports
# antenv.axon_hooks, which the container's antenv stub lacks.
try:
    _ensure_ntff_hook()
except Exception:
    pass


def kernel(**inputs) -> np.ndarray:
    out, _ = _run(inputs, trace=False)
    return out


# revision 9
# speedup vs baseline: 1.0228x; 1.0228x over previous
"""MoE (top-4 of 16 experts, SwiGLU FFN) on 8 Trainium2 NeuronCores.

Strategy: expert parallelism. The router (x @ Wr, softmax, top-4) is 0.26% of
the FLOPs and runs on host; tokens are gathered per expert on host (the
"all-to-all dispatch"), each core runs the dense SwiGLU FFN for its 2 experts
on its gathered tokens in bf16 (fp32 PSUM accumulation), and the host
scatter-adds the weighted expert outputs back ("combine").

v2 layout: stage B is computed transposed (y^T = wd^T @ h, tokens as the
moving free dim), so no token count needs 128-alignment anywhere and the
combine weight is applied by the vector engine against a host-prepared
broadcast tile (the scalar engine only runs the silus). Startup DMAs are
ordered critical-slice-first (wg f-quarter 0 + a 128-token lead chunk), the
PE is pre-warmed with dummy matmuls during the initial DMA wait (HAM clock
gate), stage B of the first two chunks is deferred one chunk so the PE never
waits on the down-proj weights, and y writeback alternates two DMA queues.

Shapes (hardcoded): B=4, S=1024, D=1024, E=16, F=512, TOPK=4. N = B*S = 4096.
All DRAM arrays are pre-tiled on host so every DMA is partition-contiguous
(128/256 descriptors of >=1 KiB).
"""

import numpy as np
import ml_dtypes

import concourse.bass as bass
import concourse.bacc as bacc
import concourse.tile as tile
from concourse import bass_utils, mybir

B, S, D = 4, 1024, 1024
E, F, TOPK = 16, 512, 4
N = B * S
NCORES = 8
EPC = E // NCORES  # experts per core
P = 128
DT = D // P  # 8
FT = F // P  # 4
TCH = 512    # token chunk (matmul moving free dim; one PSUM bank in fp32)
NWARM = 18   # PE pre-warm matmuls (N=128, run during startup DMA wait)

BF16 = ml_dtypes.bfloat16

_program_cache: dict[tuple, object] = {}


# ---------------------------------------------------------------- host router
def _route(xf: np.ndarray, Wr: np.ndarray):
    """Top-4 expert ids + renormalized weights per token.

    Renormalized top-k softmax weights == softmax over just the top-k logits,
    so the full softmax denominator is never needed.
    """
    logits = xf @ Wr  # [N, E] fp32
    idx = np.argpartition(-logits, TOPK - 1, axis=1)[:, :TOPK]  # [N, K]
    lt = np.take_along_axis(logits, idx, axis=1)
    lt = lt - lt.max(axis=1, keepdims=True)
    ex = np.exp(lt)
    w = ex / ex.sum(axis=1, keepdims=True)
    return idx, w.astype(np.float32)


def _chunks_of(C: int, ramp=(), small_tail: bool = False):
    """Split C tokens into chunks of <= TCH, as even as possible (no
    alignment requirement). Optional `ramp` chunks (e.g. 128, 320) let the PE
    start before full token blocks have arrived and keep the startup DMA
    burst small; `small_tail` puts a 128-token chunk last so the final drain
    + writeback is short."""
    sizes = []
    for r in ramp:
        if C >= r + P:
            sizes.append(r)
            C -= r
    tail = 0
    if small_tail and C >= P + P:
        tail = P
        C -= tail
    n = -(-C // TCH)
    per = C // n
    extra = C - per * n
    sizes += [per + 1] * extra + [per] * (n - extra)
    if tail:
        sizes.append(tail)
    out, t0 = [], 0
    for sz in sizes:
        out.append((t0, sz))
        t0 += sz
    return out


def _layout(caps: tuple):
    """Flat chunk descriptors shared by program build and host prep.

    Each entry: (slot, t0, tch, xt_off, gcol, y_off)
      xt_off: element offset of this chunk's [P, DT, tch] block in flat xt
      gcol:   global gathered-token column (cw / slot offset), exact
      y_off:  element offset of this chunk's 4 [P, 2*tch] pair blocks in y
    """
    slot_chunks = [
        _chunks_of(C, ramp=(P, 320) if s == 0 else (), small_tail=(s == EPC - 1))
        for s, C in enumerate(caps)
    ]
    flat = []
    xoff = yoff = 0
    gbase = 0
    for s, C in enumerate(caps):
        for (t0, tch) in slot_chunks[s]:
            flat.append((s, t0, tch, xoff, gbase + t0, yoff))
            xoff += P * DT * tch
            yoff += (DT // 2) * P * 2 * tch
        gbase += C
    return flat, xoff, yoff, gbase  # gbase == CTOT


# ---------------------------------------------------------------- device code
def _build_program(caps: tuple):
    """One SPMD program: EPC expert slots with exact capacities caps[s].

    Inputs (per core), all pre-tiled partition-major on host:
      xt  [XTELEMS]             bf16  gathered+transposed tokens, per chunk
                                      [p, d, c] blocks
      wg  [EPC, FT, P, DT, P]   bf16  wg[s, f, p, d, q] = Wg_s[d*128+p, f*128+q]
      wu  [EPC, FT, P, DT, P]   bf16
      wd  [EPC, P, FT, D]       bf16  wd[s, p, f, d] = Wd_s[f*128+p, d]
      cwr [P, CTOT]             f32   combine weight per gathered token,
                                      replicated along partitions
    Output:
      y   [YELEMS]              bf16  per chunk: 4 pair blocks [p, 2, tch]
                                      holding y^T rows (2dp+j)*128+p
    """
    flat, XTELEMS, YELEMS, CTOT = _layout(caps)
    nchunks = len(flat)

    nc = bacc.Bacc("TRN2", target_bir_lowering=False, debug=False)
    bf = mybir.dt.bfloat16
    f32 = mybir.dt.float32

    xt = nc.declare_dram_parameter("xt", [XTELEMS], bf, isOutput=False)
    wg = nc.declare_dram_parameter("wg", [EPC, FT, P, DT, P], bf, isOutput=False)
    wu = nc.declare_dram_parameter("wu", [EPC, FT, P, DT, P], bf, isOutput=False)
    wd = nc.declare_dram_parameter("wd", [EPC, P, FT, D], bf, isOutput=False)
    cwr = nc.declare_dram_parameter("cwr", [P, CTOT], f32, isOutput=False)
    y = nc.declare_dram_parameter("y", [YELEMS], bf, isOutput=True)

    with tile.TileContext(nc) as tc:
        with (
            tc.tile_pool(name="wrm", bufs=1) as wrm,
            tc.tile_pool(name="wpool", bufs=2) as wpool,
            tc.tile_pool(name="cpool", bufs=1) as cpool,
            tc.tile_pool(name="xpool", bufs=3) as xpool,
            tc.tile_pool(name="hpool", bufs=3) as hpool,
            tc.tile_pool(name="spool", bufs=2) as spool,
            tc.tile_pool(name="ypool", bufs=4) as ypool,
            tc.tile_pool(name="psA", bufs=2, space="PSUM") as psA,
            tc.tile_pool(name="psB", bufs=4, space="PSUM") as psB,
        ):
            # --- PE pre-warm: HAM releases the clock gate after ~3.4us of
            # activity; start that clock during the startup DMA wait.
            wt = wrm.tile([P, P], bf, tag="wt")
            nc.gpsimd.memset(wt[:], 0.0)
            for _ in range(NWARM):
                pw = psA.tile([P, TCH], f32, tag="pg")
                nc.tensor.matmul(
                    pw[:, :P], lhsT=wt[:], rhs=wt[:], start=True, stop=True
                )

            wg_sb = [
                wpool.tile([P, FT, DT, P], bf, tag="wg", name=f"wg_sb{s}")
                for s in range(EPC)
            ]
            wu_sb = [
                wpool.tile([P, FT, DT, P], bf, tag="wu", name=f"wu_sb{s}")
                for s in range(EPC)
            ]
            wd_sb = [
                wpool.tile([P, FT, D], bf, tag="wd", name=f"wd_sb{s}")
                for s in range(EPC)
            ]
            cw_sb = cpool.tile([P, CTOT], f32, tag="cw")

            xt_tiles = {}

            def xt_load(k, eng):
                # contiguous SBUF destination (any partial-width dst slice
                # explodes into 1024 descriptors); matmuls use a strided view
                _, _, tch, xo, _, _ = flat[k]
                t = xpool.tile([P, DT * tch], bf, tag="xt")
                src = xt[xo : xo + P * DT * tch].rearrange("(p x) -> p x", p=P)
                eng.dma_start(t[:], src)
                xt_tiles[k] = t.rearrange("p (dt c) -> p dt c", dt=DT)

            # --- startup DMAs, ordered by need time. DMA instruction issue
            # costs ~660ns of sequencer time each; the three DMA-capable
            # sequencers (gpsimd / sync / scalar) are used in parallel:
            # gpsimd carries the wg/cwr/wd stream, sync the tokens, scalar
            # the wu stream (its silus only start later).
            nc.gpsimd.dma_start(wg_sb[0][:, 0], wg[0, 0])  # first matmul gate
            xt_load(0, nc.sync)                            # 128-token lead
            nc.gpsimd.dma_start(wg_sb[0][:, 1], wg[0, 1])
            nc.gpsimd.dma_start(wg_sb[0][:, 2], wg[0, 2])
            nc.gpsimd.dma_start(wg_sb[0][:, 3], wg[0, 3])
            for f in range(FT):
                nc.scalar.dma_start(wu_sb[0][:, f], wu[0, f])
            xt_load(1, nc.sync)
            if nchunks > 2:
                xt_load(2, nc.sync)
            nc.gpsimd.dma_start(cw_sb[:], cwr[:])          # needed by 1st B
            nc.gpsimd.dma_start(wd_sb[0][:], wd[0])

            # slot-1 weight DMAs are paced into the first two stage-B blocks
            # (below): by then the startup bandwidth hump has drained.
            later_w = [
                (wg_sb[1][:, 0], wg[1, 0]),
                (wg_sb[1][:, 1], wg[1, 1]),
                (wg_sb[1][:, 2], wg[1, 2]),
                (wg_sb[1][:, 3], wg[1, 3]),
                (wu_sb[1][:, 0], wu[1, 0]),
                (wu_sb[1][:, 1], wu[1, 1]),
                (wu_sb[1][:, 2], wu[1, 2]),
                (wu_sb[1][:, 3], wu[1, 3]),
                (wd_sb[1][:], wd[1]),
            ] if EPC > 1 else []

            h_tiles = {}

            def stage_a(k):
                s, t0, tch, _, _, _ = flat[k]
                xt_sb = xt_tiles.pop(k)
                h_sb = hpool.tile([P, FT, TCH], bf, tag="h")
                sgs = []
                for f in range(FT):
                    pg = psA.tile([P, TCH], f32, tag="pg")
                    for d in range(DT):
                        nc.tensor.matmul(
                            pg[:, :tch],
                            lhsT=wg_sb[s][:, f, d, :],
                            rhs=xt_sb[:, d, :tch],
                            start=(d == 0),
                            stop=(d == DT - 1),
                        )
                    sg = spool.tile([P, TCH], f32, tag=f"sg{f}")
                    nc.scalar.activation(
                        sg[:, :tch],
                        pg[:, :tch],
                        mybir.ActivationFunctionType.Silu,
                    )
                    sgs.append(sg)
                for f in range(FT):
                    pu = psA.tile([P, TCH], f32, tag="pu")
                    for d in range(DT):
                        nc.tensor.matmul(
                            pu[:, :tch],
                            lhsT=wu_sb[s][:, f, d, :],
                            rhs=xt_sb[:, d, :tch],
                            start=(d == 0),
                            stop=(d == DT - 1),
                        )
                    nc.vector.tensor_mul(
                        out=h_sb[:, f, :tch],
                        in0=sgs[f][:, :tch],
                        in1=pu[:, :tch],
                    )
                    if f == 1 and k + 3 < nchunks:
                        xt_load(k + 3, nc.sync)
                h_tiles[k] = h_sb

            def stage_b(k, extra_w=()):
                s, t0, tch, _, gcol, yo = flat[k]
                h_sb = h_tiles.pop(k)
                extra_w = list(extra_w)
                for dp in range(DT // 2):
                    y_sb = ypool.tile([P, 2 * TCH], bf, tag="y")
                    for j in range(2):
                        dd = 2 * dp + j
                        pb = psB.tile([P, TCH], f32, tag="pb")
                        for f in range(FT):
                            nc.tensor.matmul(
                                pb[:, :tch],
                                lhsT=wd_sb[s][:, f, dd * P : (dd + 1) * P],
                                rhs=h_sb[:, f, :tch],
                                start=(f == 0),
                                stop=(f == FT - 1),
                            )
                        nc.vector.tensor_mul(
                            out=y_sb[:, j * tch : (j + 1) * tch],
                            in0=pb[:, :tch],
                            in1=cw_sb[:, gcol : gcol + tch],
                        )
                    dst = y[
                        yo + dp * P * 2 * tch : yo + (dp + 1) * P * 2 * tch
                    ].rearrange("(p x) -> p x", p=P)
                    (nc.gpsimd if dp % 2 == 0 else nc.sync).dma_start(
                        dst, y_sb[:, : 2 * tch]
                    )
                    # pace deferred weight loads between y writebacks
                    for _ in range(3):
                        if extra_w:
                            t, srcap = extra_w.pop(0)
                            nc.gpsimd.dma_start(t, srcap)

            # pipeline: defer the first stage-Bs two chunks so the PE never
            # waits on wd/cwr during the startup bandwidth hump (a PE stall
            # there also re-engages the HAM clock gate), then drain the
            # backlog so the kernel ends on a single small stage-B.
            if nchunks >= 4:
                stage_a(0)
                stage_a(1)
                stage_a(2)
                stage_b(0, extra_w=later_w[:5])
                stage_b(1, extra_w=later_w[5:])
                stage_a(3)
                stage_b(2)
                stage_b(3)
                for k in range(4, nchunks):
                    stage_a(k)
                    stage_b(k)
            elif nchunks == 3:
                stage_a(0)
                stage_a(1)
                stage_a(2)
                stage_b(0, extra_w=later_w[:5])
                stage_b(1, extra_w=later_w[5:])
                stage_b(2)
            else:
                for k in range(nchunks):
                    stage_a(k)
                for k in range(nchunks):
                    stage_b(k, extra_w=later_w if k == 0 else ())
    nc.compile()
    return nc


def _get_program(caps):
    if caps not in _program_cache:
        _program_cache[caps] = _build_program(caps)
    return _program_cache[caps]


# ------------------------------------------------------------------ profiling
def _ensure_ntff_hook():
    """The container's `antenv` stub lacks `axon_hooks`, so trn_boot's NTFF
    profile hook never gets registered and trace=True degrades to no-op.
    Register the module + ctypes hook at runtime."""
    import sys
    import types

    import antenv

    if "antenv.axon_hooks" not in sys.modules:
        mod = types.ModuleType("antenv.axon_hooks")
        mod._hook = None

        def set_axon_ntff_profile_hook(h):
            mod._hook = h

        def get_axon_ntff_profile_hook():
            return mod._hook

        mod.set_axon_ntff_profile_hook = set_axon_ntff_profile_hook
        mod.get_axon_ntff_profile_hook = get_axon_ntff_profile_hook
        sys.modules["antenv.axon_hooks"] = mod
        antenv.axon_hooks = mod
    mod = sys.modules["antenv.axon_hooks"]
    if mod._hook is None:
        from trn_agent_boot.trn_boot import _ntff_profile_via_ctypes

        mod.set_axon_ntff_profile_hook(
            _ntff_profile_via_ctypes("/opt/axon/libaxon_pjrt.so")
        )


# ---------------------------------------------------------------- entry point
def _run(inputs: dict, trace: bool = False):
    x = np.asarray(inputs["x"], dtype=np.float32)
    Wr = np.asarray(inputs["Wr"], dtype=np.float32)
    Wg = np.asarray(inputs["Wg"], dtype=np.float32)
    Wu = np.asarray(inputs["Wu"], dtype=np.float32)
    Wd = np.asarray(inputs["Wd"], dtype=np.float32)

    xf = x.reshape(N, D)
    idx, w = _route(xf, Wr)

    # group (token, weight) by expert
    flat_e = idx.ravel()
    flat_t = np.repeat(np.arange(N, dtype=np.int64), TOPK)
    flat_w = w.ravel()
    order = np.argsort(flat_e, kind="stable")
    ge, gt, gw = flat_e[order], flat_t[order], flat_w[order]
    counts = np.bincount(ge, minlength=E)
    starts = np.zeros(E + 1, dtype=np.int64)
    np.cumsum(counts, out=starts[1:])

    # global pairing: sort experts by count desc, core c gets ranks (c, 15-c);
    # slot 0 holds the larger one. caps = per-slot max over cores (exact).
    by_size = sorted(range(E), key=lambda e: -counts[e])
    slot_experts = [
        [by_size[c], by_size[E - 1 - c]] for c in range(NCORES)
    ]  # [core][slot] -> expert id
    caps = tuple(
        int(max(counts[slot_experts[c][s]] for c in range(NCORES)))
        for s in range(EPC)
    )
    flat, XTELEMS, YELEMS, CTOT = _layout(caps)
    slot_off = np.cumsum([0] + list(caps))

    xt_all = np.zeros((NCORES, XTELEMS), dtype=BF16)
    cwr_all = np.zeros((NCORES, P, CTOT), dtype=np.float32)
    wg_all = np.zeros((NCORES, EPC, FT, P, DT, P), dtype=BF16)
    wu_all = np.zeros((NCORES, EPC, FT, P, DT, P), dtype=BF16)
    wd_all = np.zeros((NCORES, EPC, P, FT, D), dtype=BF16)

    tok_lists = {}
    for c in range(NCORES):
        cw_row = np.zeros(CTOT, dtype=np.float32)
        for s in range(EPC):
            e = slot_experts[c][s]
            toks = gt[starts[e] : starts[e + 1]]
            tok_lists[(c, s)] = toks
            ne = len(toks)
            cw_row[slot_off[s] : slot_off[s] + ne] = gw[starts[e] : starts[e + 1]]
            # weights, partition-major K tiles
            wg_all[c, s] = (
                Wg[e].astype(BF16).reshape(DT, P, FT, P).transpose(2, 1, 0, 3)
            )
            wu_all[c, s] = (
                Wu[e].astype(BF16).reshape(DT, P, FT, P).transpose(2, 1, 0, 3)
            )
            wd_all[c, s] = Wd[e].astype(BF16).reshape(FT, P, D).transpose(1, 0, 2)
        cwr_all[c] = cw_row[None, :]
        # tokens, transposed + tiled per chunk: [p, d, c] = X[tok, d*128+p]
        for (s, t0, tch, xo, gcol, yo) in flat:
            toks = tok_lists[(c, s)]
            sel = toks[t0 : min(t0 + tch, len(toks))]
            blk = np.zeros((P, DT, tch), dtype=BF16)
            if len(sel):
                blk[:, :, : len(sel)] = (
                    xf[sel]
                    .astype(BF16)
                    .reshape(len(sel), DT, P)
                    .transpose(2, 1, 0)
                )
            xt_all[c, xo : xo + P * DT * tch] = blk.ravel()

    nc = _get_program(caps)
    in_maps = [
        {
            "xt": xt_all[c],
            "wg": wg_all[c],
            "wu": wu_all[c],
            "wd": wd_all[c],
            "cwr": cwr_all[c],
        }
        for c in range(NCORES)
    ]
    kwargs = {}
    if trace:
        _ensure_ntff_hook()
        kwargs = dict(trace=True, trace_cores=list(range(NCORES)))
    res = bass_utils.run_bass_kernel_spmd(
        nc, in_maps, core_ids=list(range(NCORES)), **kwargs
    )

    out = np.zeros((N, D), dtype=np.float32)
    for c in range(NCORES):
        yc = res.results[c]["y"]
        for (s, t0, tch, xo, gcol, yo) in flat:
            toks = tok_lists[(c, s)]
            sel = toks[t0 : min(t0 + tch, len(toks))]
            if not len(sel):
                continue
            blk = yc[yo : yo + (DT // 2) * P * 2 * tch].reshape(DT // 2, P, 2, tch)
            # rows (2*dp + j)*128 + p of y^T  ->  [D, tch]
            yt = blk.transpose(0, 2, 1, 3).reshape(D, tch)
            out[sel] += yt[:, : len(sel)].T.astype(np.float32)
    return out.reshape(B, S, D), res.exec_time_ns


# Pre-register the NTFF hook shim at import: if the grading harness sets
# BASS_TRACE=1, run_bass_kernel_spmd's axon trace path imports
# antenv.axon_hooks, which the container's antenv stub lacks.
try:
    _ensure_ntff_hook()
except Exception:
    pass


def kernel(**inputs) -> np.ndarray:
    out, _ = _run(inputs, trace=False)
    return out


# revision 14
# speedup vs baseline: 1.0544x; 1.0309x over previous
"""MoE (top-4 of 16 experts, SwiGLU FFN) on 8 Trainium2 NeuronCores.

Strategy: expert parallelism. The router (x @ Wr, softmax, top-4) is 0.26% of
the FLOPs and runs on host; tokens are gathered per expert on host (the
"all-to-all dispatch"), each core runs the dense SwiGLU FFN for its 2 experts
on its gathered tokens in bf16 (fp32 PSUM accumulation), and the host
scatter-adds the weighted expert outputs back ("combine").

v2 layout: stage B is computed transposed (y^T = wd^T @ h, tokens as the
moving free dim), so no token count needs 128-alignment anywhere and the
combine weight is applied by the vector engine against a host-prepared
broadcast tile (the scalar engine only runs the silus). Startup DMAs are
ordered critical-slice-first (wg f-quarter 0 + a 128-token lead chunk), the
PE is pre-warmed with dummy matmuls during the initial DMA wait (HAM clock
gate), stage B of the first two chunks is deferred one chunk so the PE never
waits on the down-proj weights, and y writeback alternates two DMA queues.

Shapes (hardcoded): B=4, S=1024, D=1024, E=16, F=512, TOPK=4. N = B*S = 4096.
All DRAM arrays are pre-tiled on host so every DMA is partition-contiguous
(128/256 descriptors of >=1 KiB).
"""

import numpy as np
import ml_dtypes

import concourse.bass as bass
import concourse.bacc as bacc
import concourse.tile as tile
from concourse import bass_utils, mybir

B, S, D = 4, 1024, 1024
E, F, TOPK = 16, 512, 4
N = B * S
NCORES = 8
EPC = E // NCORES  # experts per core
P = 128
DT = D // P  # 8
FT = F // P  # 4
TCH = 512    # token chunk (matmul moving free dim; one PSUM bank in fp32)
NWARM = 28   # PE pre-warm matmuls (N=128, run during startup DMA wait)

BF16 = ml_dtypes.bfloat16

_program_cache: dict[tuple, object] = {}


# ---------------------------------------------------------------- host router
def _route(xf: np.ndarray, Wr: np.ndarray):
    """Top-4 expert ids + renormalized weights per token.

    Renormalized top-k softmax weights == softmax over just the top-k logits,
    so the full softmax denominator is never needed.
    """
    logits = xf @ Wr  # [N, E] fp32
    idx = np.argpartition(-logits, TOPK - 1, axis=1)[:, :TOPK]  # [N, K]
    lt = np.take_along_axis(logits, idx, axis=1)
    lt = lt - lt.max(axis=1, keepdims=True)
    ex = np.exp(lt)
    w = ex / ex.sum(axis=1, keepdims=True)
    return idx, w.astype(np.float32)


def _chunks_of(C: int, ramp=(), small_tail: bool = False):
    """Split C tokens into chunks of <= TCH, as even as possible (no
    alignment requirement). Optional `ramp` chunks (e.g. 128, 320) let the PE
    start before full token blocks have arrived and keep the startup DMA
    burst small; `small_tail` puts a 128-token chunk last so the final drain
    + writeback is short."""
    sizes = []
    for r in ramp:
        if C >= r + P:
            sizes.append(r)
            C -= r
    tail = 0
    if small_tail and C >= P + P:
        tail = P
        C -= tail
    n = -(-C // TCH)
    per = C // n
    extra = C - per * n
    sizes += [per + 1] * extra + [per] * (n - extra)
    if tail:
        sizes.append(tail)
    out, t0 = [], 0
    for sz in sizes:
        out.append((t0, sz))
        t0 += sz
    return out


def _layout(caps: tuple):
    """Flat chunk descriptors shared by program build and host prep.

    Each entry: (slot, t0, tch, xt_off, gcol, y_off)
      xt_off: element offset of this chunk's [P, DT, tch] block in flat xt
      gcol:   global gathered-token column (cw / slot offset), exact
      y_off:  element offset of this chunk's 4 [P, 2*tch] pair blocks in y
    """
    slot_chunks = [
        _chunks_of(C, ramp=(P, 320) if s == 0 else (), small_tail=(s == EPC - 1))
        for s, C in enumerate(caps)
    ]
    flat = []
    xoff = yoff = 0
    gbase = 0
    for s, C in enumerate(caps):
        for (t0, tch) in slot_chunks[s]:
            flat.append((s, t0, tch, xoff, gbase + t0, yoff))
            xoff += P * DT * tch
            yoff += (DT // 2) * P * 2 * tch
        gbase += C
    return flat, xoff, yoff, gbase  # gbase == CTOT


# ---------------------------------------------------------------- device code
def _build_program(caps: tuple):
    """One SPMD program: EPC expert slots with exact capacities caps[s].

    Inputs (per core), all pre-tiled partition-major on host:
      xt  [XTELEMS]             bf16  gathered+transposed tokens, per chunk
                                      [p, d, c] blocks
      wg  [EPC, FT, P, DT, P]   bf16  wg[s, f, p, d, q] = Wg_s[d*128+p, f*128+q]
      wu  [EPC, FT, P, DT, P]   bf16
      wd  [EPC, P, FT, D]       bf16  wd[s, p, f, d] = Wd_s[f*128+p, d]
      cwr [P, CTOT]             f32   combine weight per gathered token,
                                      replicated along partitions
    Output:
      y   [YELEMS]              bf16  per chunk: 4 pair blocks [p, 2, tch]
                                      holding y^T rows (2dp+j)*128+p
    """
    flat, XTELEMS, YELEMS, CTOT = _layout(caps)
    nchunks = len(flat)

    nc = bacc.Bacc("TRN2", target_bir_lowering=False, debug=False)
    bf = mybir.dt.bfloat16
    f32 = mybir.dt.float32

    xt = nc.declare_dram_parameter("xt", [XTELEMS], bf, isOutput=False)
    wg = nc.declare_dram_parameter("wg", [EPC, FT, P, DT, P], bf, isOutput=False)
    wu = nc.declare_dram_parameter("wu", [EPC, FT, P, DT, P], bf, isOutput=False)
    wd = nc.declare_dram_parameter("wd", [EPC, P, FT, D], bf, isOutput=False)
    cwr = nc.declare_dram_parameter("cwr", [P, CTOT], f32, isOutput=False)
    y = nc.declare_dram_parameter("y", [YELEMS], bf, isOutput=True)

    with tile.TileContext(nc) as tc:
        with (
            tc.tile_pool(name="wrm", bufs=1) as wrm,
            tc.tile_pool(name="wpool", bufs=2) as wpool,
            tc.tile_pool(name="cpool", bufs=1) as cpool,
            tc.tile_pool(name="xpool", bufs=3) as xpool,
            tc.tile_pool(name="hpool", bufs=3) as hpool,
            tc.tile_pool(name="spool", bufs=2) as spool,
            tc.tile_pool(name="ypool", bufs=4) as ypool,
            tc.tile_pool(name="psA", bufs=2, space="PSUM") as psA,
            tc.tile_pool(name="psB", bufs=4, space="PSUM") as psB,
        ):
            # --- PE pre-warm: HAM releases the clock gate after ~3.4us of
            # activity; start that clock during the startup DMA wait.
            wt = wrm.tile([P, P], bf, tag="wt")
            nc.gpsimd.memset(wt[:], 0.0)
            for _ in range(NWARM):
                pw = psA.tile([P, TCH], f32, tag="pg")
                nc.tensor.matmul(
                    pw[:, :P], lhsT=wt[:], rhs=wt[:], start=True, stop=True
                )

            wg_sb = [
                wpool.tile([P, FT, DT, P], bf, tag="wg", name=f"wg_sb{s}")
                for s in range(EPC)
            ]
            wu_sb = [
                wpool.tile([P, FT, DT, P], bf, tag="wu", name=f"wu_sb{s}")
                for s in range(EPC)
            ]
            wd_sb = [
                wpool.tile([P, FT, D], bf, tag="wd", name=f"wd_sb{s}")
                for s in range(EPC)
            ]
            cw_sb = cpool.tile([P, CTOT], f32, tag="cw")

            xt_tiles = {}

            def xt_load(k, eng):
                # contiguous SBUF destination (any partial-width dst slice
                # explodes into 1024 descriptors); matmuls use a strided view
                _, _, tch, xo, _, _ = flat[k]
                t = xpool.tile([P, DT * tch], bf, tag="xt")
                src = xt[xo : xo + P * DT * tch].rearrange("(p x) -> p x", p=P)
                eng.dma_start(t[:], src)
                xt_tiles[k] = t.rearrange("p (dt c) -> p dt c", dt=DT)

            # --- startup DMAs. The DMA engines round-robin across queues
            # with outstanding descriptors, so every active queue competes
            # for HBM bandwidth immediately. To make delivery track need
            # order, the critical startup sequence is interleaved across
            # exactly two queues (sync + gpsimd) in need order; the scalar
            # queue carries nothing until the hump has drained.
            nc.gpsimd.dma_start(wg_sb[0][:, 0], wg[0, 0])  # first matmul gate
            xt_load(0, nc.sync)                            # 128-token lead
            nc.sync.dma_start(wg_sb[0][:, 1], wg[0, 1])
            nc.gpsimd.dma_start(wg_sb[0][:, 2], wg[0, 2])
            nc.sync.dma_start(wg_sb[0][:, 3], wg[0, 3])
            nc.gpsimd.dma_start(wu_sb[0][:, 0], wu[0, 0])
            nc.sync.dma_start(wu_sb[0][:, 1], wu[0, 1])
            nc.gpsimd.dma_start(wu_sb[0][:, 2], wu[0, 2])
            xt_load(1, nc.sync)
            nc.gpsimd.dma_start(wu_sb[0][:, 3], wu[0, 3])
            if nchunks > 2:
                xt_load(2, nc.sync)
            nc.gpsimd.dma_start(cw_sb[:], cwr[:])          # needed by 1st B
            nc.gpsimd.dma_start(wd_sb[0][:], wd[0])
            # slot-1 weights trail the slot-0 stream on gpsimd; DMA
            # semaphore-pool reuse paces them behind PE progress.
            if EPC > 1:
                for f in range(FT):
                    nc.gpsimd.dma_start(wg_sb[1][:, f], wg[1, f])
                for f in range(FT):
                    nc.gpsimd.dma_start(wu_sb[1][:, f], wu[1, f])
                nc.gpsimd.dma_start(wd_sb[1][:], wd[1])

            h_tiles = {}

            def stage_a(k):
                s, t0, tch, _, _, _ = flat[k]
                xt_sb = xt_tiles.pop(k)
                h_sb = hpool.tile([P, FT, TCH], bf, tag="h")
                sgs = []
                for f in range(FT):
                    pg = psA.tile([P, TCH], f32, tag="pg")
                    for d in range(DT):
                        nc.tensor.matmul(
                            pg[:, :tch],
                            lhsT=wg_sb[s][:, f, d, :],
                            rhs=xt_sb[:, d, :tch],
                            start=(d == 0),
                            stop=(d == DT - 1),
                        )
                    sg = spool.tile([P, TCH], f32, tag=f"sg{f}")
                    nc.scalar.activation(
                        sg[:, :tch],
                        pg[:, :tch],
                        mybir.ActivationFunctionType.Silu,
                    )
                    sgs.append(sg)
                for f in range(FT):
                    pu = psA.tile([P, TCH], f32, tag="pu")
                    for d in range(DT):
                        nc.tensor.matmul(
                            pu[:, :tch],
                            lhsT=wu_sb[s][:, f, d, :],
                            rhs=xt_sb[:, d, :tch],
                            start=(d == 0),
                            stop=(d == DT - 1),
                        )
                    nc.vector.tensor_mul(
                        out=h_sb[:, f, :tch],
                        in0=sgs[f][:, :tch],
                        in1=pu[:, :tch],
                    )
                    if f == 1 and k + 3 < nchunks:
                        xt_load(k + 3, nc.sync)
                h_tiles[k] = h_sb

            def stage_b(k):
                s, t0, tch, _, gcol, yo = flat[k]
                h_sb = h_tiles.pop(k)
                for dp in range(DT // 2):
                    y_sb = ypool.tile([P, 2 * TCH], bf, tag="y")
                    for j in range(2):
                        dd = 2 * dp + j
                        pb = psB.tile([P, TCH], f32, tag="pb")
                        for f in range(FT):
                            nc.tensor.matmul(
                                pb[:, :tch],
                                lhsT=wd_sb[s][:, f, dd * P : (dd + 1) * P],
                                rhs=h_sb[:, f, :tch],
                                start=(f == 0),
                                stop=(f == FT - 1),
                            )
                        nc.vector.tensor_mul(
                            out=y_sb[:, j * tch : (j + 1) * tch],
                            in0=pb[:, :tch],
                            in1=cw_sb[:, gcol : gcol + tch],
                        )
                    dst = y[
                        yo + dp * P * 2 * tch : yo + (dp + 1) * P * 2 * tch
                    ].rearrange("(p x) -> p x", p=P)
                    (nc.gpsimd if dp % 2 == 0 else nc.sync).dma_start(
                        dst, y_sb[:, : 2 * tch]
                    )

            # pipeline: defer the first stage-Bs two chunks so the PE never
            # waits on wd/cwr during the startup bandwidth hump (a PE stall
            # there also re-engages the HAM clock gate), then drain the
            # backlog so the kernel ends on a single small stage-B.
            if nchunks >= 4:
                stage_a(0)
                stage_a(1)
                stage_a(2)
                stage_b(0)
                stage_b(1)
                stage_a(3)
                stage_b(2)
                stage_b(3)
                for k in range(4, nchunks):
                    stage_a(k)
                    stage_b(k)
            else:
                for k in range(nchunks):
                    stage_a(k)
                for k in range(nchunks):
                    stage_b(k)
    nc.compile()
    return nc


def _get_program(caps):
    if caps not in _program_cache:
        _program_cache[caps] = _build_program(caps)
    return _program_cache[caps]


# ------------------------------------------------------------------ profiling
def _ensure_ntff_hook():
    """The container's `antenv` stub lacks `axon_hooks`, so trn_boot's NTFF
    profile hook never gets registered and trace=True degrades to no-op.
    Register the module + ctypes hook at runtime."""
    import sys
    import types

    import antenv

    if "antenv.axon_hooks" not in sys.modules:
        mod = types.ModuleType("antenv.axon_hooks")
        mod._hook = None

        def set_axon_ntff_profile_hook(h):
            mod._hook = h

        def get_axon_ntff_profile_hook():
            return mod._hook

        mod.set_axon_ntff_profile_hook = set_axon_ntff_profile_hook
        mod.get_axon_ntff_profile_hook = get_axon_ntff_profile_hook
        sys.modules["antenv.axon_hooks"] = mod
        antenv.axon_hooks = mod
    mod = sys.modules["antenv.axon_hooks"]
    if mod._hook is None:
        from trn_agent_boot.trn_boot import _ntff_profile_via_ctypes

        mod.set_axon_ntff_profile_hook(
            _ntff_profile_via_ctypes("/opt/axon/libaxon_pjrt.so")
        )


# ---------------------------------------------------------------- entry point
def _run(inputs: dict, trace: bool = False):
    x = np.asarray(inputs["x"], dtype=np.float32)
    Wr = np.asarray(inputs["Wr"], dtype=np.float32)
    Wg = np.asarray(inputs["Wg"], dtype=np.float32)
    Wu = np.asarray(inputs["Wu"], dtype=np.float32)
    Wd = np.asarray(inputs["Wd"], dtype=np.float32)

    xf = x.reshape(N, D)
    idx, w = _route(xf, Wr)

    # group (token, weight) by expert
    flat_e = idx.ravel()
    flat_t = np.repeat(np.arange(N, dtype=np.int64), TOPK)
    flat_w = w.ravel()
    order = np.argsort(flat_e, kind="stable")
    ge, gt, gw = flat_e[order], flat_t[order], flat_w[order]
    counts = np.bincount(ge, minlength=E)
    starts = np.zeros(E + 1, dtype=np.int64)
    np.cumsum(counts, out=starts[1:])

    # global pairing: sort experts by count desc, core c gets ranks (c, 15-c);
    # slot 0 holds the larger one. caps = per-slot max over cores (exact).
    by_size = sorted(range(E), key=lambda e: -counts[e])
    slot_experts = [
        [by_size[c], by_size[E - 1 - c]] for c in range(NCORES)
    ]  # [core][slot] -> expert id
    caps = tuple(
        int(max(counts[slot_experts[c][s]] for c in range(NCORES)))
        for s in range(EPC)
    )
    flat, XTELEMS, YELEMS, CTOT = _layout(caps)
    slot_off = np.cumsum([0] + list(caps))

    xt_all = np.zeros((NCORES, XTELEMS), dtype=BF16)
    cwr_all = np.zeros((NCORES, P, CTOT), dtype=np.float32)
    wg_all = np.zeros((NCORES, EPC, FT, P, DT, P), dtype=BF16)
    wu_all = np.zeros((NCORES, EPC, FT, P, DT, P), dtype=BF16)
    wd_all = np.zeros((NCORES, EPC, P, FT, D), dtype=BF16)

    tok_lists = {}
    for c in range(NCORES):
        cw_row = np.zeros(CTOT, dtype=np.float32)
        for s in range(EPC):
            e = slot_experts[c][s]
            toks = gt[starts[e] : starts[e + 1]]
            tok_lists[(c, s)] = toks
            ne = len(toks)
            cw_row[slot_off[s] : slot_off[s] + ne] = gw[starts[e] : starts[e + 1]]
            # weights, partition-major K tiles
            wg_all[c, s] = (
                Wg[e].astype(BF16).reshape(DT, P, FT, P).transpose(2, 1, 0, 3)
            )
            wu_all[c, s] = (
                Wu[e].astype(BF16).reshape(DT, P, FT, P).transpose(2, 1, 0, 3)
            )
            wd_all[c, s] = Wd[e].astype(BF16).reshape(FT, P, D).transpose(1, 0, 2)
        cwr_all[c] = cw_row[None, :]
        # tokens, transposed + tiled per chunk: [p, d, c] = X[tok, d*128+p]
        for (s, t0, tch, xo, gcol, yo) in flat:
            toks = tok_lists[(c, s)]
            sel = toks[t0 : min(t0 + tch, len(toks))]
            blk = np.zeros((P, DT, tch), dtype=BF16)
            if len(sel):
                blk[:, :, : len(sel)] = (
                    xf[sel]
                    .astype(BF16)
                    .reshape(len(sel), DT, P)
                    .transpose(2, 1, 0)
                )
            xt_all[c, xo : xo + P * DT * tch] = blk.ravel()

    nc = _get_program(caps)
    in_maps = [
        {
            "xt": xt_all[c],
            "wg": wg_all[c],
            "wu": wu_all[c],
            "wd": wd_all[c],
            "cwr": cwr_all[c],
        }
        for c in range(NCORES)
    ]
    kwargs = {}
    if trace:
        _ensure_ntff_hook()
        kwargs = dict(trace=True, trace_cores=list(range(NCORES)))
    res = bass_utils.run_bass_kernel_spmd(
        nc, in_maps, core_ids=list(range(NCORES)), **kwargs
    )

    out = np.zeros((N, D), dtype=np.float32)
    for c in range(NCORES):
        yc = res.results[c]["y"]
        for (s, t0, tch, xo, gcol, yo) in flat:
            toks = tok_lists[(c, s)]
            sel = toks[t0 : min(t0 + tch, len(toks))]
            if not len(sel):
                continue
            blk = yc[yo : yo + (DT // 2) * P * 2 * tch].reshape(DT // 2, P, 2, tch)
            # rows (2*dp + j)*128 + p of y^T  ->  [D, tch]
            yt = blk.transpose(0, 2, 1, 3).reshape(D, tch)
            out[sel] += yt[:, : len(sel)].T.astype(np.float32)
    return out.reshape(B, S, D), res.exec_time_ns


# Pre-register the NTFF hook shim at import: if the grading harness sets
# BASS_TRACE=1, run_bass_kernel_spmd's axon trace path imports
# antenv.axon_hooks, which the container's antenv stub lacks.
try:
    _ensure_ntff_hook()
except Exception:
    pass


def kernel(**inputs) -> np.ndarray:
    out, _ = _run(inputs, trace=False)
    return out
